# revision 1
# baseline (speedup 1.0000x reference)
"""Trainium2 Bass kernel for nn_AdvancedDocClassificationModel_46961172414521.

4-layer post-LN transformer encoder (B=8, S=512, H=1024, 32 heads x 32 dim,
FFN 4096) with learned relative-position attention bias, mean/max/first
pooling and a linear classifier head.

Strategy: pure data parallelism — B == n_cores == 8, one sequence per
NeuronCore, no collectives.  Each core runs a fused Bass/Tile program:

  - All big GEMMs in float32r (TF32-like, 11-bit mantissa, full PE rate).
  - Q/K and the attention probabilities in bf16 (validated numerically).
  - Scores are computed transposed ([key, query] layout) per head so the
    softmax denominator is a ones-matmul and P@V needs no transposes.
  - The relative-position bias Toeplitz tiles are built by overlapping-
    window DMAs from HBM with rows in reversed order (the BIR verifier
    rejects negative partition steps) and un-flipped by an anti-diagonal
    permutation matmul that accumulates directly into the scores psum.
  - LayerNorm runs sequence-major on bn_stats/bn_aggr; the residual stream
    is transposed back to feature-major with PE transposes.

The module builds and compiles the program once per process (NEFF cached on
disk); kernel() shards inputs per batch element, runs the SPMD program on
cores 0-7 and stacks the per-core [1, 20] logits.
"""

import sys
import numpy as np

sys.path.insert(0, "/opt/trn_rl_repo")

import ml_dtypes  # noqa: E402

import concourse.bass as bass  # noqa: E402
import concourse.mybir as mybir  # noqa: E402
import concourse.tile as tile  # noqa: E402
from concourse import bacc  # noqa: E402
from concourse import bass_utils  # noqa: E402
from concourse.bass import ts, ds  # noqa: E402

F32 = mybir.dt.float32
F32R = mybir.dt.float32r
BF16 = mybir.dt.bfloat16
AF = mybir.ActivationFunctionType

L, B, S, H, NH, DH, I, MAXP, NL = 4, 8, 512, 1024, 32, 32, 4096, 512, 20
EPS = 1e-12
HC = H // 128      # 8 feature chunks
SC = S // 128      # 4 sequence chunks
IC = I // 128      # 32 inner chunks
KC = S // 128      # 4 key chunks
NG = NH // 4       # 8 head groups (4 heads each = one 128-partition chunk)
ISQ = 1.0 / np.sqrt(np.float32(DH))

# ----------------------------------------------------------------------------
# program builder
# ----------------------------------------------------------------------------


def build_program(has_mask=False, has_bias=False, has_ln_affine=False):
    nc = bacc.Bacc("TRN2", target_bir_lowering=False, debug=False)

    xT_d = nc.dram_tensor("xT", (H, S), F32R, kind="ExternalInput")
    xseq_d = nc.dram_tensor("xseq", (S, H), F32, kind="ExternalInput")
    wq_d = nc.dram_tensor("wq", (L, H, H), F32R, kind="ExternalInput")
    wk_d = nc.dram_tensor("wk", (L, H, H), F32R, kind="ExternalInput")
    wv_d = nc.dram_tensor("wv", (L, H, H), F32R, kind="ExternalInput")
    wo_d = nc.dram_tensor("wo", (L, H, H), F32R, kind="ExternalInput")
    w1_d = nc.dram_tensor("w1", (L, H, I), F32R, kind="ExternalInput")
    w2_d = nc.dram_tensor("w2", (L, I, H), F32R, kind="ExternalInput")
    rel_d = nc.dram_tensor("rel", (L, NH, 2 * MAXP - 1), BF16, kind="ExternalInput")
    ones_d = nc.dram_tensor("ones32", (128, 32), BF16, kind="ExternalInput")
    eye_bf_d = nc.dram_tensor("eye_bf", (128, 128), BF16, kind="ExternalInput")
    eye_f_d = nc.dram_tensor("eye_f", (128, 128), F32, kind="ExternalInput")
    pw_d = nc.dram_tensor("pw", (3 * H, H), F32R, kind="ExternalInput")
    cw_d = nc.dram_tensor("cw", (H, NL), F32R, kind="ExternalInput")
    if has_mask:
        maskadd_d = nc.dram_tensor("maskadd", (128, KC), F32, kind="ExternalInput")
        maskbig_d = nc.dram_tensor("maskbig", (S,), F32, kind="ExternalInput")
    invn_d = nc.dram_tensor("invn", (1, 1), F32, kind="ExternalInput")
    if has_bias:
        bq_d = nc.dram_tensor("bq", (L, H), F32, kind="ExternalInput")
        bk_d = nc.dram_tensor("bk", (L, H), F32, kind="ExternalInput")
        bv_d = nc.dram_tensor("bv", (L, H), F32, kind="ExternalInput")
        bo_d = nc.dram_tensor("bo", (L, H), F32, kind="ExternalInput")
        b1_d = nc.dram_tensor("b1", (L, I), F32, kind="ExternalInput")
        b2_d = nc.dram_tensor("b2", (L, H), F32, kind="ExternalInput")
        pb_d = nc.dram_tensor("pb", (H,), F32, kind="ExternalInput")
        cb_d = nc.dram_tensor("cb", (1, NL), F32, kind="ExternalInput")
    if has_ln_affine:
        ln1g_d = nc.dram_tensor("ln1g", (L, H), F32, kind="ExternalInput")
        ln1b_d = nc.dram_tensor("ln1b", (L, H), F32, kind="ExternalInput")
        ln2g_d = nc.dram_tensor("ln2g", (L, H), F32, kind="ExternalInput")
        ln2b_d = nc.dram_tensor("ln2b", (L, H), F32, kind="ExternalInput")
        plng_d = nc.dram_tensor("plng", (1, H), F32, kind="ExternalInput")
        plnb_d = nc.dram_tensor("plnb", (1, H), F32, kind="ExternalInput")

    out_d = nc.dram_tensor("out", (1, NL), F32, kind="ExternalOutput")

    tc_cm = tile.TileContext(nc)
    tc = tc_cm.__enter__()
    import contextlib
    ctx = contextlib.ExitStack()

    ps = ctx.enter_context(tc.tile_pool(name="ps", bufs=8, space="PSUM"))
    pool_xT = ctx.enter_context(tc.tile_pool(name="xT", bufs=9))
    pool_big = ctx.enter_context(tc.tile_pool(name="big", bufs=9))
    pool_qk = ctx.enter_context(tc.tile_pool(name="qk", bufs=16))
    pool_ctx = ctx.enter_context(tc.tile_pool(name="ctxT", bufs=8))
    pool_inner = ctx.enter_context(tc.tile_pool(name="inner", bufs=18))
    pool_exp = ctx.enter_context(tc.tile_pool(name="exp", bufs=5))
    pool_w = ctx.enter_context(tc.tile_pool(name="w", bufs=3))
    pool_w1 = pool_w
    pool_rel = ctx.enter_context(tc.tile_pool(name="rel", bufs=2))
    pool_sm = ctx.enter_context(tc.tile_pool(name="sm", bufs=1))
    pool_ln = ctx.enter_context(tc.tile_pool(name="ln", bufs=2))
    pool_med = pool_sm
    pool_c = ctx.enter_context(tc.tile_pool(name="const", bufs=1))

    def psum(p=128, f=512, tag="ps"):
        return ps.tile([p, f], F32, tag=tag, name=f"ps{nc.next_id()}")

    eye_f = pool_c.tile([128, 128], F32, tag="eye_f")
    nc.sync.dma_start(eye_f[:], eye_f_d.ap()[:])
    ones_blk = pool_c.tile([128, 32], BF16, tag="ones")
    nc.sync.dma_start(ones_blk[:], ones_d.ap()[:])
    flip_bf = pool_c.tile([128, 128], BF16, tag="flip_bf")
    nc.sync.dma_start(flip_bf[:], eye_bf_d.ap()[:])
    eps_col = pool_c.tile([128, 1], F32, tag="eps")
    nc.gpsimd.memset(eps_col[:], float(EPS))
    invn_rep = pool_c.tile([128, 1], F32, tag="invn")
    nc.sync.dma_start(invn_rep[:], bass.AP(tensor=invn_d, offset=0, ap=[[0, 128], [1, 1]]))
    if has_mask:
        maskadd = pool_c.tile([128, KC], F32, tag="maskadd")
        nc.sync.dma_start(maskadd[:], maskadd_d.ap()[:])

    def bias_vec_tiles(dram_ap, n, tag):
        t = pool_c.tile([128, n], F32, tag=tag)
        for c in range(n):
            src = bass.AP(tensor=dram_ap.tensor, offset=dram_ap.offset + c * 128,
                          ap=[[1, 128], [1, 1]])
            nc.sync.dma_start(t[:, c:c + 1], src)
        return t

    xT = [pool_xT.tile([128, S], F32R, tag="xT", name=f"xT0_{c}") for c in range(HC)]
    for c in range(HC):
        nc.sync.dma_start(xT[c][:], xT_d.ap()[ts(c, 128), :])
    xseq = [pool_big.tile([128, H], F32, tag="big", name=f"xseq0_{c}") for c in range(SC)]
    for sc in range(SC):
        nc.sync.dma_start(xseq[sc][:], xseq_d.ap()[ts(sc, 128), :])

    xT_cur, xseq_cur = xT, xseq

    def proj_featmajor(w_dram_l, out_dtype, out_pool, out_tag, bias_col=None,
                       act=AF.Identity, n_out=HC):
        outs = [out_pool.tile([128, S], out_dtype, tag=out_tag,
                              name=f"{out_tag}o{nc.next_id()}_{c}") for c in range(n_out)]
        psb = {}
        for kc in range(HC):
            wt = pool_w.tile([128, 128 * n_out], F32R, tag="w", bufs=4, name=f"wt_{nc.next_id()}")
            nc.sync.dma_start(wt[:], w_dram_l[ts(kc, 128), :])
            for c in range(n_out):
                if kc == 0:
                    psb[c] = psum(tag="ps")
                nc.tensor.matmul(psb[c][:], wt[:, ts(c, 128)], xT_cur[kc][:],
                                 start=(kc == 0), stop=(kc == HC - 1))
        for c in range(n_out):
            b = bias_col[:, c:c + 1] if bias_col is not None else 0.0
            if act == AF.Identity and bias_col is None:
                nc.vector.tensor_copy(outs[c][:], psb[c][:])
            else:
                nc.scalar.activation(outs[c][:], psb[c][:], act, bias=b)
        return outs

    def proj_featmajor_grouped(w_dram_l, out_dtype, out_pool, out_tag, n_out,
                               group, bias_col=None, act=AF.Identity,
                               x_tiles=None, col_off=0):
        outs = [None] * n_out
        for g0 in range(0, n_out, group):
            gn = min(group, n_out - g0)
            psb = {}
            for kc in range(HC):
                wt = pool_w1.tile([128, 128 * group], F32R, tag="w1g",
                                  name=f"w1t_{nc.next_id()}")
                nc.sync.dma_start(wt[:, :128 * gn],
                                  w_dram_l[ts(kc, 128), ds((col_off + g0) * 128, gn * 128)])
                for c in range(gn):
                    if kc == 0:
                        psb[c] = psum(tag="ps")
                    nc.tensor.matmul(psb[c][:], wt[:, ts(c, 128)], x_tiles[kc][:],
                                     start=(kc == 0), stop=(kc == HC - 1))
            for c in range(gn):
                outs[g0 + c] = out_pool.tile([128, S], out_dtype, tag=out_tag,
                                             name=f"{out_tag}g{nc.next_id()}_{g0 + c}")
                b = (bias_col[:, col_off + g0 + c:col_off + g0 + c + 1]
                     if bias_col is not None else 0.0)
                if act == AF.Identity and bias_col is None:
                    nc.vector.tensor_copy(outs[g0 + c][:], psb[c][:])
                else:
                    nc.scalar.activation(outs[g0 + c][:], psb[c][:], act, bias=b)
        return outs

    def proj_seqmajor(w_dram_l, lhs_tiles, n_k, resid_tiles, bias_rep=None):
        outs = [pool_big.tile([128, H], F32, tag="big", name=f"sqo{nc.next_id()}_{c}")
                for c in range(SC)]
        psb = {}
        for kc in range(n_k):
            wt = pool_w.tile([128, H], F32R, tag="w", bufs=4, name=f"wt_{nc.next_id()}")
            nc.sync.dma_start(wt[:], w_dram_l[ts(kc, 128), :])
            for sc in range(SC):
                for hn in range(2):
                    if kc == 0:
                        psb[(sc, hn)] = psum(tag="ps")
                    nc.tensor.matmul(psb[(sc, hn)][:], lhs_tiles[kc][:, ts(sc, 128)],
                                     wt[:, ts(hn, 512)],
                                     start=(kc == 0), stop=(kc == n_k - 1))
        for sc in range(SC):
            for hn in range(2):
                o = outs[sc][:, ts(hn, 512)]
                if bias_rep is not None:
                    nc.vector.scalar_tensor_tensor(
                        o, psb[(sc, hn)][:], 1.0, resid_tiles[sc][:, ts(hn, 512)],
                        op0=mybir.AluOpType.mult, op1=mybir.AluOpType.add)
                    nc.vector.tensor_add(o, o, bias_rep[:, ts(hn, 512)])
                else:
                    nc.vector.tensor_add(o, psb[(sc, hn)][:],
                                         resid_tiles[sc][:, ts(hn, 512)])
        return outs

    def layernorm_seq(y_tiles, g_rep=None, b_rep=None):
        for sc in range(SC):
            st = pool_ln.tile([128, 12], F32, tag="bnst")
            nc.vector.bn_stats(st[:, 0:6], y_tiles[sc][:, 0:512])
            nc.vector.bn_stats(st[:, 6:12], y_tiles[sc][:, 512:1024])
            ag = pool_ln.tile([128, 2], F32, tag="bnag")
            nc.vector.bn_aggr(ag[:], st[:])
            sd = pool_ln.tile([128, 1], F32, tag="sd")
            nc.scalar.activation(sd[:], ag[:, 1:2], AF.Sqrt, bias=eps_col[:sd.shape[0], :])
            rs = pool_ln.tile([128, 1], F32, tag="rs")
            nc.vector.reciprocal(rs[:], sd[:])
            nmr = pool_ln.tile([128, 1], F32, tag="nmr")
            nc.vector.scalar_tensor_tensor(nmr[:], ag[:, 0:1], -1.0, rs[:],
                                           op0=mybir.AluOpType.mult,
                                           op1=mybir.AluOpType.mult)
            for hn in range(2):
                nc.scalar.activation(y_tiles[sc][:, ts(hn, 512)],
                                     y_tiles[sc][:, ts(hn, 512)], AF.Identity,
                                     bias=nmr[:], scale=rs[:])
            if g_rep is not None:
                nc.vector.tensor_mul(y_tiles[sc][:], y_tiles[sc][:], g_rep[:])
            if b_rep is not None:
                nc.vector.tensor_add(y_tiles[sc][:], y_tiles[sc][:], b_rep[:])

    def transpose_to_featmajor(seq_tiles, n_f, out_pool, out_tag):
        outs = [out_pool.tile([128, S], F32R, tag=out_tag,
                              name=f"{out_tag}t{nc.next_id()}_{c}") for c in range(n_f)]
        for hc in range(n_f):
            for sc in range(SC):
                pt = psum(tag="ps")
                nc.tensor.transpose(pt[0:128, 0:128], seq_tiles[sc][:, ts(hc, 128)],
                                    eye_f[:])
                nc.vector.tensor_copy(outs[hc][:, ts(sc, 128)], pt[0:128, 0:128])
        return outs

    def rep_row(dram_ap_row, tag):
        t = pool_w.tile([128, H], F32, tag=tag)
        src = bass.AP(tensor=dram_ap_row.tensor, offset=dram_ap_row.offset,
                      ap=[[0, 128], [1, H]])
        nc.sync.dma_start(t[:], src)
        return t

    for l in range(L):
        with nc.named_scope(f"layer{l}"):
            bqc = bkc = b1c = None
            if has_bias:
                bqc = bias_vec_tiles(bq_d.ap()[l], HC, "bqc")
                bkc = bias_vec_tiles(bk_d.ap()[l], HC, "bkc")
                b1c = bias_vec_tiles(b1_d.ap()[l], IC, "b1c")

            qT = proj_featmajor(wq_d.ap()[l], BF16, pool_qk, "qk", bias_col=bqc)
            kT = proj_featmajor(wk_d.ap()[l], BF16, pool_qk, "qk", bias_col=bkc)
            V = [pool_big.tile([128, H], BF16, tag="vbf", bufs=5, name=f"V{l}_{c}")
                 for c in range(SC)]
            psb = {}
            for kc in range(HC):
                wt = pool_w.tile([128, H], F32R, tag="w", bufs=4, name=f"wvt_{nc.next_id()}")
                nc.sync.dma_start(wt[:], wv_d.ap()[l][ts(kc, 128), :])
                for sc in range(SC):
                    for hn in range(2):
                        if kc == 0:
                            psb[(sc, hn)] = psum(tag="ps")
                        nc.tensor.matmul(psb[(sc, hn)][:], xT_cur[kc][:, ts(sc, 128)],
                                         wt[:, ts(hn, 512)],
                                         start=(kc == 0), stop=(kc == HC - 1))
            for sc in range(SC):
                for hn in range(2):
                    if has_bias:
                        bvrep = rep_row(bv_d.ap()[l], "bvrep")
                        nc.vector.tensor_add(V[sc][:, ts(hn, 512)], psb[(sc, hn)][:],
                                             bvrep[:, ts(hn, 512)])
                    else:
                        nc.vector.tensor_copy(V[sc][:, ts(hn, 512)], psb[(sc, hn)][:])

            ctxT = [pool_ctx.tile([128, S], F32R, tag="ctxT", name=f"ctxT{l}_{c}")
                    for c in range(NG)]
            for g in range(NG):
                exp_t = {}
                for kc in range(KC):
                    ps_s = {}
                    for j in range(4):
                        ps_s[j] = psum(tag="ps")
                        nc.tensor.matmul(ps_s[j][:], kT[g][ts(j, 32), ts(kc, 128)],
                                         qT[g][ts(j, 32), :],
                                         start=True, stop=False,
                                         tile_position=(32 * j, 0))
                    for j in range(4):
                        h = 4 * g + j
                        # Toeplitz rel tile, rows DMA'd in REVERSED order from
                        # HBM (overlapping ascending windows); the flip matmul
                        # (anti-diagonal permutation) un-reverses rows while
                        # accumulating into the scores psum.
                        trel = pool_rel.tile([128, S], BF16, tag="rel", bufs=4,
                                             name=f"trel{nc.next_id()}")
                        off = ((l * NH + h) * (2 * MAXP - 1)
                               + (MAXP - 1) - 128 * kc - 127)
                        srcap = bass.AP(tensor=rel_d, offset=off,
                                        ap=[[1, 128], [1, S]])
                        nc.sync.dma_start(trel[:], srcap)
                        nc.tensor.matmul(ps_s[j][:], flip_bf[:], trel[:],
                                         start=False, stop=True)
                        e = pool_exp.tile([128, S], BF16, tag="exp", bufs=17,
                                          name=f"e{nc.next_id()}")
                        mb = maskadd[:, kc:kc + 1] if has_mask else 0.0
                        nc.scalar.activation(e[:], ps_s[j][:], AF.Exp,
                                             bias=mb, scale=float(ISQ))
                        exp_t[(j, kc)] = e
                ps_ctx = psum(tag="ps")
                ps_den = psum(tag="ps")
                for j in range(4):
                    h = 4 * g + j
                    for kc in range(KC):
                        vsl = V[kc][:, ds(32 * h, 32)]
                        nc.tensor.matmul(ps_ctx[ts(j, 32), :], vsl, exp_t[(j, kc)][:],
                                         start=(kc == 0), stop=(kc == KC - 1),
                                         tile_position=(0, 32 * j))
                for j in range(4):
                    for kc in range(KC):
                        nc.tensor.matmul(ps_den[ts(j, 32), :], ones_blk[:],
                                         exp_t[(j, kc)][:],
                                         start=(kc == 0), stop=(kc == KC - 1),
                                         tile_position=(0, 32 * j))
                recip = pool_med.tile([128, S], F32, tag="recip",
                                      name=f"recip{nc.next_id()}")
                nc.vector.reciprocal(recip[:], ps_den[:])
                nc.vector.tensor_mul(ctxT[g][:], ps_ctx[:], recip[:])

            borep = rep_row(bo_d.ap()[l], "borep") if has_bias else None
            x1 = proj_seqmajor(wo_d.ap()[l], ctxT, HC, xseq_cur, bias_rep=borep)
            g1 = rep_row(ln1g_d.ap()[l], "g1rep") if has_ln_affine else None
            be1 = rep_row(ln1b_d.ap()[l], "b1rep") if has_ln_affine else None
            layernorm_seq(x1, g1, be1)
            x1T = transpose_to_featmajor(x1, HC, pool_xT, "xT")

            b2rep = rep_row(b2_d.ap()[l], "b2rep") if has_bias else None
            x2 = [pool_big.tile([128, H], F32, tag="big", name=f"x2_{l}_{c}")
                  for c in range(SC)]
            for half in range(2):
                i0 = half * (IC // 2)
                innerT = proj_featmajor_grouped(
                    w1_d.ap()[l], F32R, pool_inner, "inner", n_out=IC // 2,
                    group=8, bias_col=b1c, col_off=i0, act=AF.Gelu, x_tiles=x1T)
                psb = {}
                for ki in range(IC // 2):
                    kc = i0 + ki
                    wt = pool_w.tile([128, H], F32R, tag="w2g", name=f"w2t{l}_{kc}")
                    nc.sync.dma_start(wt[:], w2_d.ap()[l][ts(kc, 128), :])
                    for sc in range(SC):
                        for hn in range(2):
                            if ki == 0:
                                psb[(sc, hn)] = psum(tag="ps")
                            nc.tensor.matmul(psb[(sc, hn)][:],
                                             innerT[ki][:, ts(sc, 128)],
                                             wt[:, ts(hn, 512)],
                                             start=(ki == 0),
                                             stop=(ki == IC // 2 - 1))
                for sc in range(SC):
                    for hn in range(2):
                        o = x2[sc][:, ts(hn, 512)]
                        if half == 0:
                            if b2rep is not None:
                                nc.vector.scalar_tensor_tensor(
                                    o, psb[(sc, hn)][:], 1.0, x1[sc][:, ts(hn, 512)],
                                    op0=mybir.AluOpType.mult,
                                    op1=mybir.AluOpType.add)
                                nc.vector.tensor_add(o, o, b2rep[:, ts(hn, 512)])
                            else:
                                nc.vector.tensor_add(o, psb[(sc, hn)][:],
                                                     x1[sc][:, ts(hn, 512)])
                        else:
                            nc.vector.tensor_add(o, o, psb[(sc, hn)][:])
            g2 = rep_row(ln2g_d.ap()[l], "g2rep") if has_ln_affine else None
            be2 = rep_row(ln2b_d.ap()[l], "b2rep_ln") if has_ln_affine else None
            layernorm_seq(x2, g2, be2)
            x2T = transpose_to_featmajor(x2, HC, pool_xT, "xT")
            xT_cur, xseq_cur = x2T, x2

    with nc.named_scope("head"):
        pooled = pool_c.tile([128, 3 * HC], F32R, tag="pooled")
        if has_mask:
            mrep = pool_w.tile([128, S], F32, tag="mrep")
            nc.sync.dma_start(mrep[:], bass.AP(tensor=maskbig_d, offset=0,
                                               ap=[[0, 128], [1, S]]))
        for c in range(HC):
            sm = pool_sm.tile([128, 1], F32, tag="poolsum")
            nc.vector.tensor_reduce(sm[:], xT_cur[c][:], mybir.AxisListType.X,
                                    mybir.AluOpType.add)
            nc.scalar.activation(pooled[:, c:c + 1], sm[:], AF.Identity,
                                 scale=invn_rep[:])
            mx_in = xT_cur[c][:]
            if has_mask:
                tmask = pool_med.tile([128, S], F32, tag="rb")
                nc.vector.tensor_add(tmask[:], xT_cur[c][:], mrep[:])
                mx_in = tmask[:]
            mx = pool_sm.tile([128, 1], F32, tag="poolmax")
            nc.vector.tensor_reduce(mx[:], mx_in, mybir.AxisListType.X,
                                    mybir.AluOpType.max)
            nc.scalar.copy(pooled[:, HC + c:HC + c + 1], mx[:])
            nc.scalar.copy(pooled[:, 2 * HC + c:2 * HC + c + 1], xT_cur[c][:, 0:1])
        psr = {}
        for hn in range(2):
            psr[hn] = psum(tag="ps")
        for c in range(3 * HC):
            wt = pool_w.tile([128, H], F32R, tag="w", bufs=4, name=f"pwt_{c}")
            nc.sync.dma_start(wt[:], pw_d.ap()[ts(c, 128), :])
            for hn in range(2):
                nc.tensor.matmul(psr[hn][0:1, :], pooled[:, c:c + 1],
                                 wt[:, ts(hn, 512)],
                                 start=(c == 0), stop=(c == 3 * HC - 1))
        prow = pool_sm.tile([1, H], F32, tag="prow")
        for hn in range(2):
            if has_bias:
                pbrow = pool_sm.tile([1, H], F32, tag="pbrow")
                nc.sync.dma_start(pbrow[:], pb_d.ap()[None, :])
                nc.vector.tensor_add(prow[:, ts(hn, 512)], psr[hn][0:1, :],
                                     pbrow[:, ts(hn, 512)])
            else:
                nc.scalar.copy(prow[:, ts(hn, 512)], psr[hn][0:1, :])
        st = pool_sm.tile([1, 12], F32, tag="hst")
        nc.vector.bn_stats(st[:, 0:6], prow[:, 0:512])
        nc.vector.bn_stats(st[:, 6:12], prow[:, 512:1024])
        ag = pool_sm.tile([1, 2], F32, tag="hag")
        nc.vector.bn_aggr(ag[:], st[:])
        sd = pool_sm.tile([1, 1], F32, tag="hsd")
        nc.scalar.activation(sd[:], ag[:, 1:2], AF.Sqrt, bias=eps_col[:1, :])
        rs = pool_sm.tile([1, 1], F32, tag="hrs")
        nc.vector.reciprocal(rs[:], sd[:])
        nmr = pool_sm.tile([1, 1], F32, tag="hnmr")
        nc.vector.scalar_tensor_tensor(nmr[:], ag[:, 0:1], -1.0, rs[:],
                                       op0=mybir.AluOpType.mult,
                                       op1=mybir.AluOpType.mult)
        nc.scalar.activation(prow[:], prow[:], AF.Identity, bias=nmr[:], scale=rs[:])
        if has_ln_affine:
            pg = pool_sm.tile([1, H], F32, tag="hpg")
            nc.sync.dma_start(pg[:], plng_d.ap()[:])
            nc.vector.tensor_mul(prow[:], prow[:], pg[:])
            pbt = pool_sm.tile([1, H], F32, tag="hpb")
            nc.sync.dma_start(pbt[:], plnb_d.ap()[:])
            nc.vector.tensor_add(prow[:], prow[:], pbt[:])
        pcol = pool_c.tile([128, HC], F32R, tag="pcol")
        for c in range(HC):
            pt = psum(tag="ps")
            nc.tensor.transpose(pt[0:128, 0:1], prow[:, ts(c, 128)], eye_f[0:1, 0:1])
            nc.scalar.copy(pcol[:, c:c + 1], pt[0:128, 0:1])
        ps_out = psum(tag="ps")
        cwt = pool_sm.tile([128, NL * HC], F32R, tag="cwt")
        nc.sync.dma_start(cwt[:].rearrange("p (c n) -> p c n", c=HC),
                          cw_d.ap().rearrange("(c p) n -> p c n", p=128))
        for c in range(HC):
            nc.tensor.matmul(ps_out[0:1, 0:NL], pcol[:, c:c + 1],
                             cwt[:, ds(c * NL, NL)],
                             start=(c == 0), stop=(c == HC - 1))
        orow = pool_sm.tile([1, NL], F32, tag="orow")
        if has_bias:
            cbt = pool_sm.tile([1, NL], F32, tag="cbt")
            nc.sync.dma_start(cbt[:], cb_d.ap()[:])
            nc.vector.tensor_add(orow[:], ps_out[0:1, 0:NL], cbt[:])
        else:
            nc.scalar.copy(orow[:], ps_out[0:1, 0:NL])
        nc.sync.dma_start(out_d.ap()[:], orow[:])

    ctx.close()
    tc_cm.__exit__(None, None, None)
    nc.compile()
    return nc


# ----------------------------------------------------------------------------
# host-side input prep
# ----------------------------------------------------------------------------


def _prep_shared(p):
    d = {}
    d["wq"] = np.ascontiguousarray(p["Wq"], np.float32)
    d["wk"] = np.ascontiguousarray(p["Wk"], np.float32)
    d["wv"] = np.ascontiguousarray(p["Wv"], np.float32)
    d["wo"] = np.ascontiguousarray(p["Wo"], np.float32)
    d["w1"] = np.ascontiguousarray(p["W1"], np.float32)
    d["w2"] = np.ascontiguousarray(p["W2"], np.float32)
    d["rel"] = np.ascontiguousarray(
        np.transpose(np.asarray(p["rel_emb"], np.float32), (0, 2, 1))
    ).astype(ml_dtypes.bfloat16)
    d["ones32"] = np.ones((128, 32), dtype=ml_dtypes.bfloat16)
    d["eye_bf"] = np.eye(128, dtype=ml_dtypes.bfloat16)[::-1].copy()
    d["eye_f"] = np.eye(128, dtype=np.float32)
    d["pw"] = np.ascontiguousarray(p["pool_W"], np.float32)
    d["cw"] = np.ascontiguousarray(p["cls_W"], np.float32)
    return d


def _flags_for(p):
    zb = all(np.all(np.asarray(p[k]) == 0) for k in
             ("bq", "bk", "bv", "bo", "b1", "b2", "pool_b", "cls_b"))
    affine = not (all(np.all(np.asarray(p[k]) == 1) for k in
                      ("ln1_g", "ln2_g", "pool_ln_g"))
                  and all(np.all(np.asarray(p[k]) == 0) for k in
                          ("ln1_b", "ln2_b", "pool_ln_b")))
    has_mask = not np.all(np.asarray(p["attention_mask"]) == 1)
    return dict(has_mask=has_mask, has_bias=not zb, has_ln_affine=affine)


_PROGRAM_CACHE = {}


def _get_program(flags):
    key = tuple(sorted(flags.items()))
    if key not in _PROGRAM_CACHE:
        _PROGRAM_CACHE[key] = build_program(**flags)
    return _PROGRAM_CACHE[key]


def kernel(**inputs):
    p = {k: np.asarray(v) for k, v in inputs.items()}
    flags = _flags_for(p)
    nc = _get_program(flags)
    shared = _prep_shared(p)
    if flags["has_bias"]:
        shared["bq"] = np.ascontiguousarray(p["bq"], np.float32)
        shared["bk"] = np.ascontiguousarray(p["bk"], np.float32)
        shared["bv"] = np.ascontiguousarray(p["bv"], np.float32)
        shared["bo"] = np.ascontiguousarray(p["bo"], np.float32)
        shared["b1"] = np.ascontiguousarray(p["b1"], np.float32)
        shared["b2"] = np.ascontiguousarray(p["b2"], np.float32)
        shared["pb"] = np.ascontiguousarray(p["pool_b"], np.float32)
        shared["cb"] = np.ascontiguousarray(p["cls_b"], np.float32)[None, :]
    if flags["has_ln_affine"]:
        shared["ln1g"] = np.ascontiguousarray(p["ln1_g"], np.float32)
        shared["ln1b"] = np.ascontiguousarray(p["ln1_b"], np.float32)
        shared["ln2g"] = np.ascontiguousarray(p["ln2_g"], np.float32)
        shared["ln2b"] = np.ascontiguousarray(p["ln2_b"], np.float32)
        shared["plng"] = np.ascontiguousarray(p["pool_ln_g"], np.float32)[None, :]
        shared["plnb"] = np.ascontiguousarray(p["pool_ln_b"], np.float32)[None, :]

    in_maps = []
    for b in range(B):
        x = np.asarray(p["hidden_states"][b], np.float32)
        mask = np.asarray(p["attention_mask"][b], np.float32)
        m = dict(shared)
        m["xT"] = np.ascontiguousarray(x.T)
        m["xseq"] = np.ascontiguousarray(x)
        m["invn"] = np.array([[1.0 / max(mask.sum(), 1.0)]], np.float32)
        if flags["has_mask"]:
            madd = (1.0 - mask) * np.float32(-1e9)
            m["maskadd"] = np.ascontiguousarray(madd.reshape(KC, 128).T)
            m["maskbig"] = np.ascontiguousarray(madd)
        in_maps.append(m)

    res = bass_utils.run_bass_kernel_spmd(nc, in_maps, core_ids=list(range(B)))
    out = np.concatenate([res.results[b]["out"] for b in range(B)], axis=0)
    return out.astype(np.float32)



# revision 10
# speedup vs baseline: 1.2694x; 1.2694x over previous
"""Trainium2 Bass kernel for nn_AdvancedDocClassificationModel_46961172414521.

4-layer post-LN transformer encoder (B=8, S=512, H=1024, 32 heads x 32 dim,
FFN 4096) with learned relative-position attention bias, mean/max/first
pooling and a linear classifier head.

Strategy: pure data parallelism — B == n_cores == 8, one sequence per
NeuronCore, no collectives.  Each core runs a fused Bass/Tile program:

  - All big GEMMs in float16 (11-bit mantissa like tf32, half the HBM/SBUF
    traffic of f32r; magnitudes here are O(1) so fp16 range is ample).
  - Scores are computed transposed ([key, query] layout) per head so the
    softmax denominator is a ones-matmul and P@V needs no transposes.
  - The learned relative-position bias is added WITHOUT materializing
    Toeplitz tiles through the PE: each head's (reversed, isq-prescaled)
    1023-long bias vector is DMA'd once as an overlapping-window tile
    [128, 896] (ascending row starts, valid AP), and a DVE
    scalar_tensor_tensor with a NEGATIVE free stride reads the diagonal
    windows while fusing the 1/sqrt(dh) score scaling.
  - LayerNorm runs sequence-major on bn_stats/bn_aggr; the residual stream
    stays fp32; PE transposes rebuild the feature-major fp16 copy.

The module builds and compiles the program once per process (NEFF cached on
disk); kernel() shards inputs per batch element, runs the SPMD program on
cores 0-7 and stacks the per-core [1, 20] logits.
"""

import sys
import numpy as np

sys.path.insert(0, "/opt/trn_rl_repo")

import ml_dtypes  # noqa: E402,F401

import concourse.bass as bass  # noqa: E402
import concourse.mybir as mybir  # noqa: E402
import concourse.tile as tile  # noqa: E402
from concourse import bacc  # noqa: E402
from concourse import bass_utils  # noqa: E402
from concourse.bass import ts, ds  # noqa: E402

F32 = mybir.dt.float32
F16 = mybir.dt.float16
AF = mybir.ActivationFunctionType

L, B, S, H, NH, DH, I, MAXP, NL = 4, 8, 512, 1024, 32, 32, 4096, 512, 20
EPS = 1e-12
HC = H // 128      # 8 feature chunks
SC = S // 128      # 4 sequence chunks
IC = I // 128      # 32 inner chunks
KC = S // 128      # 4 key chunks
NG = NH // 4       # 8 head groups (4 heads each = one 128-partition chunk)
ISQ = 1.0 / np.sqrt(np.float32(DH))
RELW = 896         # rel window cols: covers all 4 key chunks per head

# ----------------------------------------------------------------------------
# program builder
# ----------------------------------------------------------------------------


def build_program(has_mask=False, has_bias=False, has_ln_affine=False):
    nc = bacc.Bacc("TRN2", target_bir_lowering=False, debug=False)

    xT_d = nc.dram_tensor("xT", (H, S), F16, kind="ExternalInput")
    xseq_d = nc.dram_tensor("xseq", (S, H), F32, kind="ExternalInput")
    wq_d = nc.dram_tensor("wq", (L, H, H), F16, kind="ExternalInput")
    wk_d = nc.dram_tensor("wk", (L, H, H), F16, kind="ExternalInput")
    wv_d = nc.dram_tensor("wv", (L, H, H), F16, kind="ExternalInput")
    wo_d = nc.dram_tensor("wo", (L, H, H), F16, kind="ExternalInput")
    w1_d = nc.dram_tensor("w1", (L, H, I), F16, kind="ExternalInput")
    w2_d = nc.dram_tensor("w2", (L, I, H), F16, kind="ExternalInput")
    # reversed exp(isq*rel) vectors, padded to 1024 per (layer, head)
    relw_d = nc.dram_tensor("relw", (L, NH, 1024), F16, kind="ExternalInput")
    eye_f_d = nc.dram_tensor("eye_f", (128, 128), F32, kind="ExternalInput")
    pw_d = nc.dram_tensor("pw", (3 * H, H), F16, kind="ExternalInput")
    cw_d = nc.dram_tensor("cw", (H, NL), F16, kind="ExternalInput")
    if has_mask:
        maskadd_d = nc.dram_tensor("maskadd", (128, KC), F32, kind="ExternalInput")
        maskbig_d = nc.dram_tensor("maskbig", (S,), F32, kind="ExternalInput")
    invn_d = nc.dram_tensor("invn", (1, 1), F32, kind="ExternalInput")
    if has_bias:
        bq_d = nc.dram_tensor("bq", (L, H), F32, kind="ExternalInput")
        bk_d = nc.dram_tensor("bk", (L, H), F32, kind="ExternalInput")
        bv_d = nc.dram_tensor("bv", (L, H), F32, kind="ExternalInput")
        bo_d = nc.dram_tensor("bo", (L, H), F32, kind="ExternalInput")
        b1_d = nc.dram_tensor("b1", (L, I), F32, kind="ExternalInput")
        b2_d = nc.dram_tensor("b2", (L, H), F32, kind="ExternalInput")
        pb_d = nc.dram_tensor("pb", (H,), F32, kind="ExternalInput")
        cb_d = nc.dram_tensor("cb", (1, NL), F32, kind="ExternalInput")
    if has_ln_affine:
        ln1g_d = nc.dram_tensor("ln1g", (L, H), F32, kind="ExternalInput")
        ln1b_d = nc.dram_tensor("ln1b", (L, H), F32, kind="ExternalInput")
        ln2g_d = nc.dram_tensor("ln2g", (L, H), F32, kind="ExternalInput")
        ln2b_d = nc.dram_tensor("ln2b", (L, H), F32, kind="ExternalInput")
        plng_d = nc.dram_tensor("plng", (1, H), F32, kind="ExternalInput")
        plnb_d = nc.dram_tensor("plnb", (1, H), F32, kind="ExternalInput")

    out_d = nc.dram_tensor("out", (1, NL), F32, kind="ExternalOutput")

    tc_cm = tile.TileContext(nc)
    tc = tc_cm.__enter__()
    import contextlib
    ctx = contextlib.ExitStack()

    ps = ctx.enter_context(tc.tile_pool(name="ps", bufs=8, space="PSUM"))
    pool_xT = ctx.enter_context(tc.tile_pool(name="xT", bufs=9))
    pool_big = ctx.enter_context(tc.tile_pool(name="big", bufs=9))
    pool_qk = ctx.enter_context(tc.tile_pool(name="qk", bufs=16))
    pool_ctx = ctx.enter_context(tc.tile_pool(name="ctxT", bufs=8))
    pool_inner = ctx.enter_context(tc.tile_pool(name="inner", bufs=18))
    pool_exp = ctx.enter_context(tc.tile_pool(name="exp", bufs=5))
    pool_w = ctx.enter_context(tc.tile_pool(name="w", bufs=3))
    pool_w1 = pool_w
    pool_rel = ctx.enter_context(tc.tile_pool(name="rel", bufs=2))
    pool_sm = ctx.enter_context(tc.tile_pool(name="sm", bufs=1))
    pool_ln = ctx.enter_context(tc.tile_pool(name="ln", bufs=2))
    pool_med = pool_sm
    pool_c = ctx.enter_context(tc.tile_pool(name="const", bufs=1))

    def psum(p=128, f=512, tag="ps"):
        return ps.tile([p, f], F32, tag=tag, name=f"ps{nc.next_id()}")

    eye_f = pool_c.tile([128, 128], F32, tag="eye_f")
    nc.sync.dma_start(eye_f[:], eye_f_d.ap()[:])
    eps_col = pool_c.tile([128, 1], F32, tag="eps")
    nc.gpsimd.memset(eps_col[:], float(EPS))
    invn_rep = pool_c.tile([128, 1], F32, tag="invn")
    nc.sync.dma_start(invn_rep[:], bass.AP(tensor=invn_d, offset=0, ap=[[0, 128], [1, 1]]))
    if has_mask:
        maskadd = pool_c.tile([128, KC], F32, tag="maskadd")
        nc.sync.dma_start(maskadd[:], maskadd_d.ap()[:])

    # Persistent "fat" P@V stationaries: per key-chunk a [128, NH*128] fp16
    # tile; head h = 4*hh + j owns cols [128h, 128h+128) with its V block at
    # +32j and a ones block at +32*((j+2)%4) (zeros elsewhere, set once).
    # One matmul per (head, key-chunk) then yields ctx AND the softmax
    # denominator in disjoint psum partitions, pair-parity arranged so the
    # reciprocal/multiply stay partition-aligned.
    v2 = [pool_c.tile([128, NH * 128], F16, tag=f"v2_{kc}", name=f"v2_{kc}")
          for kc in range(KC)]
    for kc in range(KC):
        nc.gpsimd.memset(v2[kc][:], 0.0)
        v2v = v2[kc][:].rearrange("p (hh j c) -> p hh j c", j=4, c=128)
        for j in range(4):
            o = 32 * ((j + 2) % 4)
            nc.gpsimd.memset(v2v[:, :, j, o:o + 32], 1.0)

    def bias_vec_tiles(dram_ap, n, tag):
        t = pool_c.tile([128, n], F32, tag=tag)
        for c in range(n):
            src = bass.AP(tensor=dram_ap.tensor, offset=dram_ap.offset + c * 128,
                          ap=[[1, 128], [1, 1]])
            nc.sync.dma_start(t[:, c:c + 1], src)
        return t

    xT = [pool_xT.tile([128, S], F16, tag="xT", name=f"xT0_{c}") for c in range(HC)]
    for c in range(HC):
        nc.sync.dma_start(xT[c][:], xT_d.ap()[ts(c, 128), :])
    xseq = [pool_big.tile([128, H], F32, tag="big", name=f"xseq0_{c}") for c in range(SC)]
    for sc in range(SC):
        nc.sync.dma_start(xseq[sc][:], xseq_d.ap()[ts(sc, 128), :])

    xT_cur, xseq_cur = xT, xseq

    def proj_featmajor(w_dram_l, out_dtype, out_pool, out_tag, bias_col=None,
                       act=AF.Identity, n_out=HC):
        outs = [out_pool.tile([128, S], out_dtype, tag=out_tag,
                              name=f"{out_tag}o{nc.next_id()}_{c}") for c in range(n_out)]
        psb = {}
        for kc in range(HC):
            wt = pool_w.tile([128, 128 * n_out], F16, tag="w", bufs=4, name=f"wt_{nc.next_id()}")
            nc.sync.dma_start(wt[:], w_dram_l[ts(kc, 128), :])
            for c in range(n_out):
                if kc == 0:
                    psb[c] = psum(tag="ps")
                nc.tensor.matmul(psb[c][:], wt[:, ts(c, 128)], xT_cur[kc][:],
                                 start=(kc == 0), stop=(kc == HC - 1))
        for c in range(n_out):
            b = bias_col[:, c:c + 1] if bias_col is not None else 0.0
            if act == AF.Identity and bias_col is None:
                nc.vector.tensor_copy(outs[c][:], psb[c][:])
            else:
                nc.scalar.activation(outs[c][:], psb[c][:], act, bias=b)
        return outs

    def proj_featmajor_grouped(w_dram_l, out_dtype, out_pool, out_tag, n_out,
                               group, bias_col=None, act=AF.Identity,
                               x_tiles=None, col_off=0):
        outs = [None] * n_out
        for g0 in range(0, n_out, group):
            gn = min(group, n_out - g0)
            psb = {}
            for kc in range(HC):
                wt = pool_w1.tile([128, 128 * group], F16, tag="w1g",
                                  name=f"w1t_{nc.next_id()}")
                nc.sync.dma_start(wt[:, :128 * gn],
                                  w_dram_l[ts(kc, 128), ds((col_off + g0) * 128, gn * 128)])
                for c in range(gn):
                    if kc == 0:
                        psb[c] = psum(tag="ps")
                    nc.tensor.matmul(psb[c][:], wt[:, ts(c, 128)], x_tiles[kc][:],
                                     start=(kc == 0), stop=(kc == HC - 1))
            for c in range(gn):
                outs[g0 + c] = out_pool.tile([128, S], out_dtype, tag=out_tag,
                                             name=f"{out_tag}g{nc.next_id()}_{g0 + c}")
                b = (bias_col[:, col_off + g0 + c:col_off + g0 + c + 1]
                     if bias_col is not None else 0.0)
                if act == AF.Identity and bias_col is None:
                    nc.vector.tensor_copy(outs[g0 + c][:], psb[c][:])
                else:
                    nc.scalar.activation(outs[g0 + c][:], psb[c][:], act, bias=b)
        return outs

    def proj_seqmajor(w_dram_l, lhs_tiles, n_k, resid_tiles, bias_rep=None):
        outs = [pool_big.tile([128, H], F32, tag="big", name=f"sqo{nc.next_id()}_{c}")
                for c in range(SC)]
        psb = {}
        for kc in range(n_k):
            wt = pool_w.tile([128, H], F16, tag="w", bufs=4, name=f"wt_{nc.next_id()}")
            nc.sync.dma_start(wt[:], w_dram_l[ts(kc, 128), :])
            for sc in range(SC):
                for hn in range(2):
                    if kc == 0:
                        psb[(sc, hn)] = psum(tag="ps")
                    nc.tensor.matmul(psb[(sc, hn)][:], lhs_tiles[kc][:, ts(sc, 128)],
                                     wt[:, ts(hn, 512)],
                                     start=(kc == 0), stop=(kc == n_k - 1))
        for sc in range(SC):
            for hn in range(2):
                o = outs[sc][:, ts(hn, 512)]
                if bias_rep is not None:
                    nc.vector.scalar_tensor_tensor(
                        o, psb[(sc, hn)][:], 1.0, resid_tiles[sc][:, ts(hn, 512)],
                        op0=mybir.AluOpType.mult, op1=mybir.AluOpType.add)
                    nc.vector.tensor_add(o, o, bias_rep[:, ts(hn, 512)])
                else:
                    nc.vector.tensor_add(o, psb[(sc, hn)][:],
                                         resid_tiles[sc][:, ts(hn, 512)])
        return outs

    def layernorm_seq(y_tiles, g_rep=None, b_rep=None):
        for sc in range(SC):
            st = pool_ln.tile([128, 12], F32, tag="bnst")
            nc.vector.bn_stats(st[:, 0:6], y_tiles[sc][:, 0:512])
            nc.vector.bn_stats(st[:, 6:12], y_tiles[sc][:, 512:1024])
            ag = pool_ln.tile([128, 2], F32, tag="bnag")
            nc.vector.bn_aggr(ag[:], st[:])
            sd = pool_ln.tile([128, 1], F32, tag="sd")
            nc.scalar.activation(sd[:], ag[:, 1:2], AF.Sqrt, bias=eps_col[:sd.shape[0], :])
            rs = pool_ln.tile([128, 1], F32, tag="rs")
            nc.vector.reciprocal(rs[:], sd[:])
            nmr = pool_ln.tile([128, 1], F32, tag="nmr")
            nc.vector.scalar_tensor_tensor(nmr[:], ag[:, 0:1], -1.0, rs[:],
                                           op0=mybir.AluOpType.mult,
                                           op1=mybir.AluOpType.mult)
            for hn in range(2):
                nc.scalar.activation(y_tiles[sc][:, ts(hn, 512)],
                                     y_tiles[sc][:, ts(hn, 512)], AF.Identity,
                                     bias=nmr[:], scale=rs[:])
            if g_rep is not None:
                nc.vector.tensor_mul(y_tiles[sc][:], y_tiles[sc][:], g_rep[:])
            if b_rep is not None:
                nc.vector.tensor_add(y_tiles[sc][:], y_tiles[sc][:], b_rep[:])

    def transpose_to_featmajor(seq_tiles, n_f, out_pool, out_tag):
        outs = [out_pool.tile([128, S], F16, tag=out_tag,
                              name=f"{out_tag}t{nc.next_id()}_{c}") for c in range(n_f)]
        for hc in range(n_f):
            for sc in range(SC):
                pt = psum(tag="ps")
                nc.tensor.transpose(pt[0:128, 0:128], seq_tiles[sc][:, ts(hc, 128)],
                                    eye_f[:])
                nc.vector.tensor_copy(outs[hc][:, ts(sc, 128)], pt[0:128, 0:128])
        return outs

    def rep_row(dram_ap_row, tag):
        t = pool_w.tile([128, H], F32, tag=tag)
        src = bass.AP(tensor=dram_ap_row.tensor, offset=dram_ap_row.offset,
                      ap=[[0, 128], [1, H]])
        nc.sync.dma_start(t[:], src)
        return t

    for l in range(L):
        with nc.named_scope(f"layer{l}"):
            bqc = bkc = b1c = None
            if has_bias:
                bqc = bias_vec_tiles(bq_d.ap()[l], HC, "bqc")
                bkc = bias_vec_tiles(bk_d.ap()[l], HC, "bkc")
                b1c = bias_vec_tiles(b1_d.ap()[l], IC, "b1c")

            qT = proj_featmajor(wq_d.ap()[l], F16, pool_qk, "qk", bias_col=bqc)
            kT = proj_featmajor(wk_d.ap()[l], F16, pool_qk, "qk", bias_col=bkc)
            psb = {}
            for kc in range(HC):
                wt = pool_w.tile([128, H], F16, tag="w", bufs=4, name=f"wvt_{nc.next_id()}")
                nc.sync.dma_start(wt[:], wv_d.ap()[l][ts(kc, 128), :])
                for sc in range(SC):
                    for hn in range(2):
                        if kc == 0:
                            psb[(sc, hn)] = psum(tag="ps")
                        nc.tensor.matmul(psb[(sc, hn)][:], xT_cur[kc][:, ts(sc, 128)],
                                         wt[:, ts(hn, 512)],
                                         start=(kc == 0), stop=(kc == HC - 1))
            for sc in range(SC):
                for hn in range(2):
                    src = psb[(sc, hn)][:]
                    if has_bias:
                        bvrep = rep_row(bv_d.ap()[l], "bvrep")
                        tmpv = pool_med.tile([128, 512], F32, tag="tmpv",
                                             name=f"tmpv{nc.next_id()}")
                        nc.vector.tensor_add(tmpv[:], src,
                                             bvrep[:, ts(hn, 512)])
                        src = tmpv[:]
                    # scatter each head's 32 V cols into its fat-block slot:
                    # head h=4*hh+j -> v2 col 128*h + 32*j (j-stride 160)
                    v2ap = v2[sc][:]
                    dst = bass.AP(tensor=v2ap.tensor,
                                  offset=v2ap.offset + 2048 * hn,
                                  ap=[[v2ap.ap[0][0], 128], [512, 4], [160, 4],
                                      [1, 32]])
                    nc.vector.tensor_copy(
                        dst, src.rearrange("p (hh j c) -> p hh j c", j=4, c=32))

            ctxT = [pool_ctx.tile([128, S], F16, tag="ctxT", name=f"ctxT{l}_{c}")
                    for c in range(NG)]
            for g in range(NG):
                # per-head overlapping-window exp(isq*rel) tiles (row r holds
                # the reversed vector starting at element r)
                wins = []
                for j in range(4):
                    h = 4 * g + j
                    wt_rel = pool_rel.tile([128, RELW], F16, tag="rel", bufs=6,
                                           name=f"relw{nc.next_id()}")
                    src = bass.AP(tensor=relw_d, offset=(l * NH + h) * 1024,
                                  ap=[[1, 128], [1, RELW]])
                    nc.sync.dma_start(wt_rel[:], src)
                    wins.append(wt_rel)
                exp_t = {}
                for kc in range(KC):
                    for j in range(4):
                        ps_s = psum(tag="ps")
                        nc.tensor.matmul(ps_s[:], kT[g][ts(j, 32), ts(kc, 128)],
                                         qT[g][ts(j, 32), :],
                                         start=True, stop=True,
                                         tile_position=(32 * j, 0))
                        eraw = pool_exp.tile([128, S], F16, tag="eraw", bufs=6,
                                             name=f"eraw{nc.next_id()}")
                        mb = maskadd[:, kc:kc + 1] if has_mask else 0.0
                        nc.scalar.activation(eraw[:], ps_s[:], AF.Exp, bias=mb,
                                             scale=float(ISQ))
                        # multiplicative rel bias via negative-stride window
                        wap = wins[j][:]
                        negsl = bass.AP(tensor=wap.tensor,
                                        offset=wap.offset + 511 + 128 * kc,
                                        ap=[[wap.ap[0][0], 128], [-1, 512]])
                        e = pool_exp.tile([128, S], F16, tag="exp", bufs=17,
                                          name=f"e{nc.next_id()}")
                        nc.vector.tensor_mul(e[:], eraw[:], negsl)
                        exp_t[(j, kc)] = e
                psv = [psum(tag="ps"), psum(tag="ps")]
                for kc in range(KC):
                    for j in range(4):
                        h = 4 * g + j
                        nc.tensor.matmul(psv[j // 2][:], v2[kc][:, ds(128 * h, 128)],
                                         exp_t[(j, kc)][:],
                                         start=(kc == 0 and j % 2 == 0),
                                         stop=(kc == KC - 1 and j % 2 == 1))
                for p in range(2):
                    # psv[0] = [ctx0|ctx1|den0|den1], psv[1] = [den2|den3|ctx2|ctx3]
                    den_sl = ds(64, 64) if p == 0 else ds(0, 64)
                    ctx_sl = ds(0, 64) if p == 0 else ds(64, 64)
                    rec = pool_med.tile([128, S], F32, tag="recip",
                                        name=f"recip{nc.next_id()}")
                    nc.vector.reciprocal(rec[den_sl, :], psv[p][den_sl, :])
                    # partition-shift the replicated denominators onto the
                    # ctx partitions (SBUF->SBUF DMA crosses partitions)
                    nc.sync.dma_start(rec[ctx_sl, :], rec[den_sl, :])
                    nc.vector.tensor_mul(ctxT[g][ctx_sl, :], psv[p][ctx_sl, :],
                                         rec[ctx_sl, :])

            borep = rep_row(bo_d.ap()[l], "borep") if has_bias else None
            x1 = proj_seqmajor(wo_d.ap()[l], ctxT, HC, xseq_cur, bias_rep=borep)
            g1 = rep_row(ln1g_d.ap()[l], "g1rep") if has_ln_affine else None
            be1 = rep_row(ln1b_d.ap()[l], "b1rep") if has_ln_affine else None
            layernorm_seq(x1, g1, be1)
            x1T = transpose_to_featmajor(x1, HC, pool_xT, "xT")

            b2rep = rep_row(b2_d.ap()[l], "b2rep") if has_bias else None
            x2 = [pool_big.tile([128, H], F32, tag="big", name=f"x2_{l}_{c}")
                  for c in range(SC)]
            for half in range(2):
                i0 = half * (IC // 2)
                innerT = proj_featmajor_grouped(
                    w1_d.ap()[l], F16, pool_inner, "inner", n_out=IC // 2,
                    group=8, bias_col=b1c, col_off=i0, act=AF.Gelu, x_tiles=x1T)
                psb = {}
                for ki in range(IC // 2):
                    kc = i0 + ki
                    wt = pool_w.tile([128, H], F16, tag="w2g", name=f"w2t{l}_{kc}")
                    nc.sync.dma_start(wt[:], w2_d.ap()[l][ts(kc, 128), :])
                    for sc in range(SC):
                        for hn in range(2):
                            if ki == 0:
                                psb[(sc, hn)] = psum(tag="ps")
                            nc.tensor.matmul(psb[(sc, hn)][:],
                                             innerT[ki][:, ts(sc, 128)],
                                             wt[:, ts(hn, 512)],
                                             start=(ki == 0),
                                             stop=(ki == IC // 2 - 1))
                for sc in range(SC):
                    for hn in range(2):
                        o = x2[sc][:, ts(hn, 512)]
                        if half == 0:
                            if b2rep is not None:
                                nc.vector.scalar_tensor_tensor(
                                    o, psb[(sc, hn)][:], 1.0, x1[sc][:, ts(hn, 512)],
                                    op0=mybir.AluOpType.mult,
                                    op1=mybir.AluOpType.add)
                                nc.vector.tensor_add(o, o, b2rep[:, ts(hn, 512)])
                            else:
                                nc.vector.tensor_add(o, psb[(sc, hn)][:],
                                                     x1[sc][:, ts(hn, 512)])
                        else:
                            nc.vector.tensor_add(o, o, psb[(sc, hn)][:])
            g2 = rep_row(ln2g_d.ap()[l], "g2rep") if has_ln_affine else None
            be2 = rep_row(ln2b_d.ap()[l], "b2rep_ln") if has_ln_affine else None
            layernorm_seq(x2, g2, be2)
            x2T = transpose_to_featmajor(x2, HC, pool_xT, "xT")
            xT_cur, xseq_cur = x2T, x2

    with nc.named_scope("head"):
        pooled = pool_c.tile([128, 3 * HC], F16, tag="pooled")
        if has_mask:
            mrep = pool_w.tile([128, S], F32, tag="mrep")
            nc.sync.dma_start(mrep[:], bass.AP(tensor=maskbig_d, offset=0,
                                               ap=[[0, 128], [1, S]]))
        for c in range(HC):
            sm = pool_sm.tile([128, 1], F32, tag="poolsum")
            nc.vector.tensor_reduce(sm[:], xT_cur[c][:], mybir.AxisListType.X,
                                    mybir.AluOpType.add)
            nc.scalar.activation(pooled[:, c:c + 1], sm[:], AF.Identity,
                                 scale=invn_rep[:])
            mx_in = xT_cur[c][:]
            if has_mask:
                tmask = pool_med.tile([128, S], F32, tag="rb")
                nc.vector.tensor_add(tmask[:], xT_cur[c][:], mrep[:])
                mx_in = tmask[:]
            mx = pool_sm.tile([128, 1], F32, tag="poolmax")
            nc.vector.tensor_reduce(mx[:], mx_in, mybir.AxisListType.X,
                                    mybir.AluOpType.max)
            nc.scalar.copy(pooled[:, HC + c:HC + c + 1], mx[:])
            nc.scalar.copy(pooled[:, 2 * HC + c:2 * HC + c + 1], xT_cur[c][:, 0:1])
        psr = {}
        for hn in range(2):
            psr[hn] = psum(tag="ps")
        for c in range(3 * HC):
            wt = pool_w.tile([128, H], F16, tag="w", bufs=4, name=f"pwt_{c}")
            nc.sync.dma_start(wt[:], pw_d.ap()[ts(c, 128), :])
            for hn in range(2):
                nc.tensor.matmul(psr[hn][0:1, :], pooled[:, c:c + 1],
                                 wt[:, ts(hn, 512)],
                                 start=(c == 0), stop=(c == 3 * HC - 1))
        prow = pool_sm.tile([1, H], F32, tag="prow")
        for hn in range(2):
            if has_bias:
                pbrow = pool_sm.tile([1, H], F32, tag="pbrow")
                nc.sync.dma_start(pbrow[:], pb_d.ap()[None, :])
                nc.vector.tensor_add(prow[:, ts(hn, 512)], psr[hn][0:1, :],
                                     pbrow[:, ts(hn, 512)])
            else:
                nc.scalar.copy(prow[:, ts(hn, 512)], psr[hn][0:1, :])
        st = pool_sm.tile([1, 12], F32, tag="hst")
        nc.vector.bn_stats(st[:, 0:6], prow[:, 0:512])
        nc.vector.bn_stats(st[:, 6:12], prow[:, 512:1024])
        ag = pool_sm.tile([1, 2], F32, tag="hag")
        nc.vector.bn_aggr(ag[:], st[:])
        sd = pool_sm.tile([1, 1], F32, tag="hsd")
        nc.scalar.activation(sd[:], ag[:, 1:2], AF.Sqrt, bias=eps_col[:1, :])
        rs = pool_sm.tile([1, 1], F32, tag="hrs")
        nc.vector.reciprocal(rs[:], sd[:])
        nmr = pool_sm.tile([1, 1], F32, tag="hnmr")
        nc.vector.scalar_tensor_tensor(nmr[:], ag[:, 0:1], -1.0, rs[:],
                                       op0=mybir.AluOpType.mult,
                                       op1=mybir.AluOpType.mult)
        nc.scalar.activation(prow[:], prow[:], AF.Identity, bias=nmr[:], scale=rs[:])
        if has_ln_affine:
            pg = pool_sm.tile([1, H], F32, tag="hpg")
            nc.sync.dma_start(pg[:], plng_d.ap()[:])
            nc.vector.tensor_mul(prow[:], prow[:], pg[:])
            pbt = pool_sm.tile([1, H], F32, tag="hpb")
            nc.sync.dma_start(pbt[:], plnb_d.ap()[:])
            nc.vector.tensor_add(prow[:], prow[:], pbt[:])
        pcol = pool_c.tile([128, HC], F16, tag="pcol")
        for c in range(HC):
            pt = psum(tag="ps")
            nc.tensor.transpose(pt[0:128, 0:1], prow[:, ts(c, 128)], eye_f[0:1, 0:1])
            nc.scalar.copy(pcol[:, c:c + 1], pt[0:128, 0:1])
        ps_out = psum(tag="ps")
        cwt = pool_sm.tile([128, NL * HC], F16, tag="cwt")
        nc.sync.dma_start(cwt[:].rearrange("p (c n) -> p c n", c=HC),
                          cw_d.ap().rearrange("(c p) n -> p c n", p=128))
        for c in range(HC):
            nc.tensor.matmul(ps_out[0:1, 0:NL], pcol[:, c:c + 1],
                             cwt[:, ds(c * NL, NL)],
                             start=(c == 0), stop=(c == HC - 1))
        orow = pool_sm.tile([1, NL], F32, tag="orow")
        if has_bias:
            cbt = pool_sm.tile([1, NL], F32, tag="cbt")
            nc.sync.dma_start(cbt[:], cb_d.ap()[:])
            nc.vector.tensor_add(orow[:], ps_out[0:1, 0:NL], cbt[:])
        else:
            nc.scalar.copy(orow[:], ps_out[0:1, 0:NL])
        nc.sync.dma_start(out_d.ap()[:], orow[:])

    ctx.close()
    tc_cm.__exit__(None, None, None)
    nc.compile()
    return nc


# ----------------------------------------------------------------------------
# host-side input prep
# ----------------------------------------------------------------------------


def _prep_shared(p):
    d = {}
    d["wq"] = np.ascontiguousarray(p["Wq"], np.float32).astype(np.float16)
    d["wk"] = np.ascontiguousarray(p["Wk"], np.float32).astype(np.float16)
    d["wv"] = np.ascontiguousarray(p["Wv"], np.float32).astype(np.float16)
    d["wo"] = np.ascontiguousarray(p["Wo"], np.float32).astype(np.float16)
    d["w1"] = np.ascontiguousarray(p["W1"], np.float32).astype(np.float16)
    d["w2"] = np.ascontiguousarray(p["W2"], np.float32).astype(np.float16)
    # reversed exp(isq*rel) vectors: relw[l, h, i] = exp(isq*rel[l, 1022-i, h])
    rel = np.asarray(p["rel_emb"], np.float32)  # [L, 2*MAXP-1, DH]
    relw = np.ones((L, NH, 1024), np.float32)
    relw[:, :, :2 * MAXP - 1] = np.exp(
        np.transpose(rel[:, ::-1, :], (0, 2, 1)) * float(ISQ))
    d["relw"] = relw.astype(np.float16)
    d["eye_f"] = np.eye(128, dtype=np.float32)
    d["pw"] = np.ascontiguousarray(p["pool_W"], np.float32).astype(np.float16)
    d["cw"] = np.ascontiguousarray(p["cls_W"], np.float32).astype(np.float16)
    return d


def _flags_for(p):
    zb = all(np.all(np.asarray(p[k]) == 0) for k in
             ("bq", "bk", "bv", "bo", "b1", "b2", "pool_b", "cls_b"))
    affine = not (all(np.all(np.asarray(p[k]) == 1) for k in
                      ("ln1_g", "ln2_g", "pool_ln_g"))
                  and all(np.all(np.asarray(p[k]) == 0) for k in
                          ("ln1_b", "ln2_b", "pool_ln_b")))
    has_mask = not np.all(np.asarray(p["attention_mask"]) == 1)
    return dict(has_mask=has_mask, has_bias=not zb, has_ln_affine=affine)


_PROGRAM_CACHE = {}


def _get_program(flags):
    key = tuple(sorted(flags.items()))
    if key not in _PROGRAM_CACHE:
        _PROGRAM_CACHE[key] = build_program(**flags)
    return _PROGRAM_CACHE[key]


def _build_in_maps(p, flags, shared):
    in_maps = []
    for b in range(B):
        x = np.asarray(p["hidden_states"][b], np.float32)
        mask = np.asarray(p["attention_mask"][b], np.float32)
        m = dict(shared)
        m["xT"] = np.ascontiguousarray(x.T).astype(np.float16)
        m["xseq"] = np.ascontiguousarray(x)
        m["invn"] = np.array([[1.0 / max(mask.sum(), 1.0)]], np.float32)
        if flags["has_mask"]:
            madd = (1.0 - mask) * np.float32(-1e9)
            m["maskadd"] = np.ascontiguousarray(madd.reshape(KC, 128).T)
            m["maskbig"] = np.ascontiguousarray(madd)
        in_maps.append(m)
    return in_maps


def kernel(**inputs):
    p = {k: np.asarray(v) for k, v in inputs.items()}
    flags = _flags_for(p)
    nc = _get_program(flags)
    shared = _prep_shared(p)
    if flags["has_bias"]:
        shared["bq"] = np.ascontiguousarray(p["bq"], np.float32)
        shared["bk"] = np.ascontiguousarray(p["bk"], np.float32)
        shared["bv"] = np.ascontiguousarray(p["bv"], np.float32)
        shared["bo"] = np.ascontiguousarray(p["bo"], np.float32)
        shared["b1"] = np.ascontiguousarray(p["b1"], np.float32)
        shared["b2"] = np.ascontiguousarray(p["b2"], np.float32)
        shared["pb"] = np.ascontiguousarray(p["pool_b"], np.float32)
        shared["cb"] = np.ascontiguousarray(p["cls_b"], np.float32)[None, :]
    if flags["has_ln_affine"]:
        shared["ln1g"] = np.ascontiguousarray(p["ln1_g"], np.float32)
        shared["ln1b"] = np.ascontiguousarray(p["ln1_b"], np.float32)
        shared["ln2g"] = np.ascontiguousarray(p["ln2_g"], np.float32)
        shared["ln2b"] = np.ascontiguousarray(p["ln2_b"], np.float32)
        shared["plng"] = np.ascontiguousarray(p["pool_ln_g"], np.float32)[None, :]
        shared["plnb"] = np.ascontiguousarray(p["pool_ln_b"], np.float32)[None, :]

    in_maps = _build_in_maps(p, flags, shared)
    res = bass_utils.run_bass_kernel_spmd(nc, in_maps, core_ids=list(range(B)))
    out = np.concatenate([res.results[b]["out"] for b in range(B)], axis=0)
    return out.astype(np.float32)
